# revision 5
# baseline (speedup 1.0000x reference)
"""CrossAttention TRN2 Bass kernel.

Problem: out[b] = softmax((q[b] @ Wq.T) @ (k[b] @ Wk.T).T) @ (v[b] @ Wv.T)
  q/k/v: [8, 2048, 512] f32, Wq/Wk/Wv: [512, 512] f32.

Sharding: data-parallel over batch -- core b computes batch b entirely.

Per-core pipeline (all matmuls contract over the SBUF partition dim):
  A. PE-transpose Wq/Wk/Wv -> WT[d, e] tiles (f32r transpose = exact permutation).
  B. PE-transpose query/key/value -> xT[d, i]; project:
       q'^T[e,i], k'^T[e,j]  (fp32 matmuls: scores feed exp, which amplifies
       absolute score error ~exp; fp32 keeps it ~1e-5)
       v'[j,d']              (stored f32r: feeds the f32r output matmul)
  C. stream over 16 query blocks:
       scores chunk [128,512] = q'^T.T @ k'^T     (fp32, PSUM)
       row max via reduce_max(negate) + min-combine
       exp(scores - max) on ACT with accum_out -> denominator
       PE-transpose exp weights -> wT [j, i] (f32r via DVE-rounding copy)
       out [128,512] = wT.T @ v'                  (f32r matmul: 11-bit operand
       rounding only perturbs the final convex combination, ~4e-4 of scale)
       scale rows by 1/den during PSUM->SBUF copy, DMA out.
"""
import sys

if "/opt/trn_rl_repo" not in sys.path:
    sys.path.insert(0, "/opt/trn_rl_repo")

import numpy as np

import concourse.bacc as bacc
import concourse.mybir as mybir
import concourse.tile as tile
from concourse.bass_utils import run_bass_kernel_spmd
from concourse.masks import make_identity

F32 = mybir.dt.float32
F32R = mybir.dt.float32r
AX = mybir.AxisListType.X
ALU = mybir.AluOpType
EXP = mybir.ActivationFunctionType.Exp

B, NQ, NK, D = 8, 2048, 2048, 512
P = 128
NIB = NQ // P   # query blocks
NJB = NK // P   # key blocks
NDB = D // P    # feature blocks
JC = 512        # scores j-chunk width (one PSUM bank of fp32)
NJC = NK // JC
IC = 512        # projection i-chunk width
NIC = NQ // IC

_CACHE = {}


def _build():
    nc = bacc.Bacc("TRN2", target_bir_lowering=False)
    q_d = nc.dram_tensor("query", [NQ, D], F32, kind="ExternalInput")
    k_d = nc.dram_tensor("key", [NK, D], F32, kind="ExternalInput")
    v_d = nc.dram_tensor("value", [NK, D], F32, kind="ExternalInput")
    w_d = {
        "wq": nc.dram_tensor("wq", [D, D], F32, kind="ExternalInput"),
        "wk": nc.dram_tensor("wk", [D, D], F32, kind="ExternalInput"),
        "wv": nc.dram_tensor("wv", [D, D], F32, kind="ExternalInput"),
    }
    out_d = nc.dram_tensor("out", [NQ, D], F32, kind="ExternalOutput")

    with tile.TileContext(nc) as tc:
        with tc.tile_pool(name="persist", bufs=1) as pp:
            ident_f = pp.tile([P, P], F32, tag="ident_f")
            make_identity(nc, ident_f[:])

            # WT[w][db][d_local, e] == W[e, db*128 + d_local]
            WT = {
                (w, db): pp.tile([P, D], F32, tag=f"WT_{w}_{db}", name=f"WT_{w}_{db}")
                for w in ("wq", "wk", "wv")
                for db in range(NDB)
            }
            # q'^T / k'^T: [e_local, i] per e-block; v': [j_local, d'] per j-block
            qpT = [pp.tile([P, NQ], F32, tag=f"qpT{eb}", name=f"qpT{eb}") for eb in range(NDB)]
            kpT = [pp.tile([P, NK], F32, tag=f"kpT{eb}", name=f"kpT{eb}") for eb in range(NDB)]
            vp = [pp.tile([P, D], F32R, tag=f"vp{jb}", name=f"vp{jb}") for jb in range(NJB)]

            # ---------------- Phase A+B: weights, input transposes, projections
            with (
                tc.tile_pool(name="stage", bufs=2) as sp,
                tc.tile_pool(name="xTp", bufs=1) as xp,
                tc.tile_pool(name="psT", bufs=3, space="PSUM") as ps_t,
                tc.tile_pool(name="psP", bufs=3, space="PSUM") as ps_p,
            ):
                for w in ("wq", "wk", "wv"):
                    wnat = sp.tile([P, NDB, D], F32, tag="wnat")
                    nc.sync.dma_start(
                        wnat[:], w_d[w].rearrange("(a p) d -> p a d", p=P)
                    )
                    for a in range(NDB):        # e-block of W rows
                        for db in range(NDB):   # d-block (columns)
                            pt = ps_t.tile([P, P], F32, tag="pt")
                            nc.tensor.transpose(
                                pt[:],
                                wnat[:, a, db * P : (db + 1) * P],
                                ident_f[:],
                            )
                            nc.any.tensor_copy(
                                WT[(w, db)][:, a * P : (a + 1) * P],
                                pt[:],
                            )

                for tname, xd, w in (("q", q_d, "wq"), ("k", k_d, "wk"), ("v", v_d, "wv")):
                    xT = [xp.tile([P, NQ], F32, tag=f"xT{db}", name=f"xT{db}") for db in range(NDB)]
                    xre = xd.rearrange("(n p) d -> p n d", p=P)
                    for g in range(4):  # 4 pieces of 4 row-blocks each
                        xnat = sp.tile([P, 4, D], F32, tag="xnat")
                        nc.sync.dma_start(xnat[:], xre[:, 4 * g : 4 * g + 4, :])
                        for nb in range(4):
                            ib = 4 * g + nb
                            for db in range(NDB):
                                pt = ps_t.tile([P, P], F32, tag="pt")
                                nc.tensor.transpose(
                                    pt[:],
                                    xnat[:, nb, db * P : (db + 1) * P],
                                    ident_f[:],
                                )
                                nc.any.tensor_copy(
                                    xT[db][:, ib * P : (ib + 1) * P],
                                    pt[:],
                                )
                    if tname in ("q", "k"):
                        dst = qpT if tname == "q" else kpT
                        for eb in range(NDB):
                            for ic in range(NIC):
                                pm = ps_p.tile([P, IC], F32, tag="pm")
                                for db in range(NDB):
                                    nc.tensor.matmul(
                                        pm[:],
                                        WT[(w, db)][:, eb * P : (eb + 1) * P],
                                        xT[db][:, ic * IC : (ic + 1) * IC],
                                        start=(db == 0),
                                        stop=(db == NDB - 1),
                                    )
                                nc.any.tensor_copy(
                                    dst[eb][:, ic * IC : (ic + 1) * IC], pm[:]
                                )
                    else:
                        for jb in range(NJB):
                            pm = ps_p.tile([P, D], F32, tag="pm")
                            for db in range(NDB):
                                nc.tensor.matmul(
                                    pm[:],
                                    xT[db][:, jb * P : (jb + 1) * P],
                                    WT[(w, db)][:],
                                    start=(db == 0),
                                    stop=(db == NDB - 1),
                                )
                            # F32R destination: DVE copy rounds -> valid f32r operand
                            nc.any.tensor_copy(vp[jb][:], pm[:])

            # ---------------- Phase C: attention, streamed over query blocks
            with (
                tc.tile_pool(name="cs", bufs=2) as cs,
                tc.tile_pool(name="stat", bufs=2) as st,
                tc.tile_pool(name="psS", bufs=5, space="PSUM") as ps_s,
                tc.tile_pool(name="psT2", bufs=2, space="PSUM") as ps_t2,
                tc.tile_pool(name="psO", bufs=1, space="PSUM") as ps_o,
            ):
                for ib in range(NIB):
                    schunks = []
                    for jc in range(NJC):
                        sc = ps_s.tile([P, JC], F32, tag="sc")
                        for eb in range(NDB):
                            nc.tensor.matmul(
                                sc[:],
                                qpT[eb][:, ib * P : (ib + 1) * P],
                                kpT[eb][:, jc * JC : (jc + 1) * JC],
                                start=(eb == 0),
                                stop=(eb == NDB - 1),
                            )
                        schunks.append(sc)

                    nmax = []
                    for jc in range(NJC):
                        nm = st.tile([P, 1], F32, tag=f"nm{jc}")
                        nc.vector.reduce_max(
                            nm[:], schunks[jc][:], axis=AX, negate=True
                        )
                        nmax.append(nm)
                    nm01 = st.tile([P, 1], F32, tag="nm01")
                    nc.vector.tensor_tensor(nm01[:], nmax[0][:], nmax[1][:], op=ALU.min)
                    nm23 = st.tile([P, 1], F32, tag="nm23")
                    nc.vector.tensor_tensor(nm23[:], nmax[2][:], nmax[3][:], op=ALU.min)
                    nmall = st.tile([P, 1], F32, tag="nmall")
                    nc.vector.tensor_tensor(nmall[:], nm01[:], nm23[:], op=ALU.min)

                    w_sb = cs.tile([P, NK], F32, tag="w")
                    dchunk = []
                    for jc in range(NJC):
                        dc = st.tile([P, 1], F32, tag=f"dc{jc}", name=f"dc{jc}")
                        nc.scalar.activation(
                            w_sb[:, jc * JC : (jc + 1) * JC],
                            schunks[jc][:],
                            EXP,
                            bias=nmall[:],
                            scale=1.0,
                            accum_out=dc[:],  # accum_out holds THIS chunk's row-sum
                        )
                        dchunk.append(dc)
                    d01 = st.tile([P, 1], F32, tag="d01")
                    nc.vector.tensor_tensor(d01[:], dchunk[0][:], dchunk[1][:], op=ALU.add)
                    d23 = st.tile([P, 1], F32, tag="d23")
                    nc.vector.tensor_tensor(d23[:], dchunk[2][:], dchunk[3][:], op=ALU.add)
                    den = st.tile([P, 1], F32, tag="den")
                    nc.vector.tensor_tensor(den[:], d01[:], d23[:], op=ALU.add)
                    rinv = st.tile([P, 1], F32, tag="rinv")
                    nc.vector.reciprocal(rinv[:], den[:])

                    wT = cs.tile([P, NK], F32R, tag="wT")  # [j_local, js*128 + i_local]
                    for js in range(NJB):
                        pt2 = ps_t2.tile([P, P], F32, tag="pt2")
                        nc.tensor.transpose(
                            pt2[:], w_sb[:, js * P : (js + 1) * P], ident_f[:]
                        )
                        nc.any.tensor_copy(wT[:, js * P : (js + 1) * P], pt2[:])

                    po = ps_o.tile([P, D], F32, tag="po")
                    for js in range(NJB):
                        nc.tensor.matmul(
                            po[:],
                            wT[:, js * P : (js + 1) * P],
                            vp[js][:],
                            start=(js == 0),
                            stop=(js == NJB - 1),
                        )
                    ob = cs.tile([P, D], F32, tag="ob")
                    nc.vector.tensor_scalar_mul(ob[:], po[:], rinv[:])
                    nc.sync.dma_start(out_d[ib * P : (ib + 1) * P, :], ob[:])

    nc.compile()
    return nc


def _get_nc():
    if "nc" not in _CACHE:
        _CACHE["nc"] = _build()
    return _CACHE["nc"]


def kernel(query, key, value, Wq, Wk, Wv, _trace=False):
    query = np.ascontiguousarray(np.asarray(query, dtype=np.float32))
    key = np.ascontiguousarray(np.asarray(key, dtype=np.float32))
    value = np.ascontiguousarray(np.asarray(value, dtype=np.float32))
    Wq = np.ascontiguousarray(np.asarray(Wq, dtype=np.float32))
    Wk = np.ascontiguousarray(np.asarray(Wk, dtype=np.float32))
    Wv = np.ascontiguousarray(np.asarray(Wv, dtype=np.float32))

    nc = _get_nc()
    in_maps = [
        {
            "query": query[b],
            "key": key[b],
            "value": value[b],
            "wq": Wq,
            "wk": Wk,
            "wv": Wv,
        }
        for b in range(B)
    ]
    res = run_bass_kernel_spmd(nc, in_maps, list(range(B)), trace=_trace)
    out = np.stack([res.results[b]["out"] for b in range(B)]).astype(np.float32)
    if _trace:
        _CACHE["last_result"] = res
    return out
